# revision 7
# baseline (speedup 1.0000x reference)
"""Trainium2 Bass kernel for nn_DepthAttention (depth attention over d=32).

v2 design (from trace analysis of the 502us baseline):
  - PE was 89% occupied but at 35% MFU: every matmul paid a serialized
    ~107ns LDWEIGHTS, and the k-projection ran bf16.  Fixes:
      * k-projection in fp8 e4m3 DoubleRow (contraction 256+64 as two
        DR matmuls, 0.5 cyc/row) with host-packed planar-pair panels.
      * 2-nt-group PSUM tiles ([128,1024] = 2 banks) so each stationary
        serves 2 matmuls back-to-back.
  - d-major free layout (d outer, pix innermost) so the DVE broadcast
    operands have stride-1 innermost dims -> 2x_1P eligibility for the
    v*attn multiply and the fold tree.
  - Engine rebalance: DVE reads k-PSUM directly (fused drain+mul, 1x),
    ACT drains v (DVE v*attn then runs 2x from SBUF bf16), GPSIMD takes
    fold steps 2-5, exp stays on ACT.  Attn broadcast + outputs ride the
    scalar HWDGE ring; ctx loads ride the sync ring.
  - q-projection for all 8 blocks runs once upfront (also serves as the
    HAM warmup); wk is scaled x16 into fp8 range, compensated in wq.

Sharding: h (64) split across 8 cores -> 8 rows of h per core, no halo.
Per core: 1024 pixels in 8 blocks of P=128.  Softmax max-subtraction is
skipped (logits are O(1)).  The denominator comes free: a constant
ones-channel in the bf16 ctx panel makes the v-projection's chunk-2
matmul emit rows 64:72 = raw attn, whose d-fold is sum_d exp(sim).
"""

import sys

sys.path.insert(0, "/opt/trn_rl_repo")

from contextlib import ExitStack  # noqa: E402

import ml_dtypes  # noqa: E402
import numpy as np  # noqa: E402

import concourse.bacc as bacc  # noqa: E402
import concourse.bass as bass  # noqa: E402
import concourse.mybir as mybir  # noqa: E402
import concourse.tile as tile  # noqa: E402

HEADS = 8
DH = 40
CIN = 320
INNER = HEADS * DH  # 320
D = 32
B = 2
H = 64
W = 64
NCORES = 8
HLOC = H // NCORES  # 8
PIX_B = HLOC * W  # 512
P = 128
NBLK = B * PIX_B // P  # 8
DP = D * P  # 4096
NT = DP // 512  # 8
SCALE = DH ** -0.5
WK_SC = 16.0  # wk is scaled into fp8 range; compensated in wq

F32 = mybir.dt.float32
F32R = mybir.dt.float32r
BF16 = mybir.dt.bfloat16
FP8 = mybir.dt.float8e4
NPBF = ml_dtypes.bfloat16
NPF8 = ml_dtypes.float8_e4m3  # TRN FP8_EXP4: bias 7, max +-240
DR = mybir.MatmulPerfMode.DoubleRow

CHUNKS = [(0, 128), (128, 128), (256, 64)]
# v-projection output sizes: chunk2 carries 8 extra denominator rows
VSZ = [128, 128, 72]


def _head_of(c):
    return c // DH


def _bcast_runs(o0, nv):
    """Row-replication runs for broadcasting attn head-rows into a chunk's
    channel rows: list of (row0, head0, n_heads, reps_per_head)."""
    runs, r = [], 0
    while r < nv:
        c = o0 + r
        h = c // DH
        run = min((h + 1) * DH - c, nv - r)
        if run == DH:
            nh = 1
            while (r + (nh + 1) * DH <= nv and (o0 + r + nh * DH) % DH == 0):
                nh += 1
            runs.append((r, h, nh, DH))
            r += nh * DH
        else:
            runs.append((r, h, 1, run))
            r += run
    return runs


def make_constants():
    sel = np.zeros((128, 24), NPBF)
    for mo, (c0, csz) in enumerate(CHUNKS):
        for r in range(csz):
            sel[r, mo * 8 + _head_of(c0 + r)] = 1.0
    rsel = np.zeros((8, 384), np.float32)
    for mo, (c0, csz) in enumerate(CHUNKS):
        for r in range(csz):
            rsel[_head_of(c0 + r), mo * 128 + r] = 1.0
    return sel, rsel


def to_fp8(x):
    return np.clip(x, -240.0, 240.0).astype(NPF8)


def pack_weight_T(w, ones_cols=False):
    """w [out, in] -> bf16 packed lhsT [128, 3*M] with M = out (+8 den
    cols when ones_cols).  Chunk kc of the 'in' dim at free offset kc*M;
    chunk 2 gets an extra contraction row 64 (the ones-channel), wired to
    the 8 denominator columns when ones_cols."""
    wt = np.ascontiguousarray(w.T, dtype=np.float32)  # [in, out]
    od = wt.shape[1]
    m = od + 8 if ones_cols else od
    p = np.zeros((128, 3 * m), NPBF)
    for kc, (c0, csz) in enumerate(CHUNKS):
        p[0:csz, kc * m:kc * m + od] = wt[c0:c0 + csz, :]
    if ones_cols:
        for n in range(8):
            p[64, 2 * m + od + n] = 1.0  # ones-channel -> den col n (chunk2)
    return p


def pack_wk_fp8(wk):
    """wk [320, 320] -> DoubleRow planar-pair stationaries.
    main [128, 2*320]: [p, j*320+m] = wk_sc[m, j*128+p]  (c = j*128+p)
    tail [32, 2*320]:  [p, j*320+m] = wk_sc[m, 256+j*32+p]"""
    wk_sc = np.asarray(wk, np.float32) * WK_SC
    main = np.zeros((128, 640), NPF8)
    tailp = np.zeros((32, 640), NPF8)
    for j in range(2):
        main[:, j * 320:(j + 1) * 320] = to_fp8(wk_sc[:, j * 128:(j + 1) * 128].T)
        tailp[:, j * 320:(j + 1) * 320] = to_fp8(
            wk_sc[:, 256 + j * 32:256 + (j + 1) * 32].T)
    return main, tailp


def build_nc():
    nc = bacc.Bacc(
        "TRN2",
        target_bir_lowering=False,
        debug=False,
        enable_asserts=False,
        num_devices=NCORES,
    )

    ctx_t = nc.dram_tensor("ctx", [NBLK, CIN + 1, DP], BF16, kind="ExternalInput")
    c8m_t = nc.dram_tensor("c8m", [NBLK, 128, 2 * DP], FP8, kind="ExternalInput")
    c8t_t = nc.dram_tensor("c8t", [NBLK, 32, 2 * DP], FP8, kind="ExternalInput")
    s8d_t = nc.dram_tensor("s8d", [2, 8, DP], BF16, kind="Internal")
    x_t = nc.dram_tensor("x", [CIN, NBLK * P], BF16, kind="ExternalInput")
    wq_t = nc.dram_tensor("wq_p", [128, 960], BF16, kind="ExternalInput")
    wk8m_t = nc.dram_tensor("wk8m_p", [128, 640], FP8, kind="ExternalInput")
    wk8t_t = nc.dram_tensor("wk8t_p", [32, 640], FP8, kind="ExternalInput")
    wv_t = nc.dram_tensor("wv_p", [128, 984], BF16, kind="ExternalInput")
    wo_t = nc.dram_tensor("wo_p", [128, 960], BF16, kind="ExternalInput")
    sel_t = nc.dram_tensor("sel_p", [128, 24], BF16, kind="ExternalInput")
    rsel_t = nc.dram_tensor("rsel_p", [8, 384], F32R, kind="ExternalInput")
    bout_t = nc.dram_tensor("bout_p", [128, 3], F32, kind="ExternalInput")
    out_t = nc.dram_tensor("out", [B, INNER, HLOC, W], F32, kind="ExternalOutput")

    ctx_ap = ctx_t.ap()
    out_ap = out_t.ap()

    with tile.TileContext(nc) as tc, ExitStack() as ctxs:
        ep = ctxs.enter_context

        const_pool = ep(tc.tile_pool(name="const", bufs=1))
        qx_pool = ep(tc.tile_pool(name="qx", bufs=1))
        ctx_pool = ep(tc.tile_pool(name="ctxp", bufs=6))
        c8m_pool = ep(tc.tile_pool(name="c8mp", bufs=2))
        c8t_pool = ep(tc.tile_pool(name="c8tp", bufs=2))
        tmp_pool = ep(tc.tile_pool(name="tmpp", bufs=6))
        s8_pool = ep(tc.tile_pool(name="s8p", bufs=2))
        ebc_pool = ep(tc.tile_pool(name="ebcp", bufs=4))
        vpd_pool = ep(tc.tile_pool(name="vpdp", bufs=4))
        mv_pool = ep(tc.tile_pool(name="mvp", bufs=3))
        fs_pool = ep(tc.tile_pool(name="fsp", bufs=2))
        ov_pool = ep(tc.tile_pool(name="ovp", bufs=2))
        sm_pool = ep(tc.tile_pool(name="smp", bufs=2))
        y_pool = ep(tc.tile_pool(name="yp", bufs=2))

        pp_pool = ep(tc.tile_pool(name="pp", bufs=3, space="PSUM"))
        sp_pool = ep(tc.tile_pool(name="sp", bufs=1, space="PSUM"))
        mp_pool = ep(tc.tile_pool(name="mp", bufs=1, space="PSUM"))

        # ---- constants ----
        wk8m_sb = const_pool.tile([128, 640], FP8, tag="wk8m")
        wk8t_sb = const_pool.tile([32, 640], FP8, tag="wk8t")
        wq_sb = const_pool.tile([128, 960], BF16, tag="wq")
        wv_sb = const_pool.tile([128, 984], BF16, tag="wv")
        wo_sb = const_pool.tile([128, 960], BF16, tag="wo")
        sel_sb = const_pool.tile([128, 24], BF16, tag="sel")
        rsel_sb = const_pool.tile([128, 384], F32R, tag="rsel")
        bout_sb = const_pool.tile([128, 3], F32, tag="bout")
        x_sb = qx_pool.tile([128, 3 * NBLK * P], BF16, tag="xall")
        q_sb = qx_pool.tile([128, 3 * NBLK * P], BF16, tag="qall")

        for kc, (c0, csz) in enumerate(CHUNKS):
            nc.sync.dma_start(x_sb[0:csz, kc * 1024:(kc + 1) * 1024],
                              x_t.ap()[c0:c0 + csz, :])
        nc.sync.dma_start(wq_sb[:], wq_t.ap())
        nc.sync.dma_start(wk8m_sb[:], wk8m_t.ap())
        nc.sync.dma_start(wk8t_sb[:], wk8t_t.ap())
        nc.sync.dma_start(wv_sb[:], wv_t.ap())
        nc.sync.dma_start(wo_sb[:], wo_t.ap())
        nc.sync.dma_start(sel_sb[:], sel_t.ap())
        nc.sync.dma_start(rsel_sb[64:72, :], rsel_t.ap())
        nc.sync.dma_start(bout_sb[:], bout_t.ap())

        wk8m_v = wk8m_sb[:].rearrange("p (j m) -> p j m", j=2)
        wk8t_v = wk8t_sb[:].rearrange("p (j m) -> p j m", j=2)

        # ---- q projection for all 8 blocks (also HAM warmup) ----
        # q layout: q_sb [128, (mo, blk, pix)]
        for mo, (o0, osz) in enumerate(CHUNKS):
            qp = pp_pool.tile([128, 1024], F32, tag="pp")
            for kc, (c0, csz) in enumerate(CHUNKS):
                for half in range(2):
                    nc.tensor.matmul(
                        qp[0:osz, half * 512:(half + 1) * 512],
                        wq_sb[0:csz, kc * 320 + o0:kc * 320 + o0 + osz],
                        x_sb[0:csz, kc * 1024 + half * 512:kc * 1024 + (half + 1) * 512],
                        start=(kc == 0), stop=(kc == 2),
                    )
            nc.scalar.activation(q_sb[0:osz, mo * 1024:(mo + 1) * 1024],
                                 qp[0:osz, :],
                                 mybir.ActivationFunctionType.Copy)

        def phase_a(blk):
            """ctx DMA, fp8-DR k proj, k*q (DVE from PSUM), selector
            reduce, exp, attn broadcast via DRAM bounce."""
            c8m = c8m_pool.tile([128, 2 * DP], FP8, tag="c8m")
            nc.sync.dma_start(c8m[:], c8m_t.ap()[blk])
            c8t = c8t_pool.tile([32, 2 * DP], FP8, tag="c8t")
            nc.sync.dma_start(c8t[:], c8t_t.ap()[blk])
            ctx_sb = []
            for kc, (c0, csz) in enumerate(CHUNKS):
                t = ctx_pool.tile([128, DP], BF16, tag="ctx")
                ksz = csz + 1 if kc == 2 else csz  # chunk2 + ones-channel
                nc.sync.dma_start(t[0:ksz, :], ctx_ap[blk, c0:c0 + ksz, :])
                ctx_sb.append(t)

            c8m_v = c8m[:].rearrange("p (j f) -> p j f", j=2)
            c8t_v = c8t[:].rearrange("p (j f) -> p j f", j=2)
            s8 = s8_pool.tile([8, DP], BF16, tag="s8")
            tmp_tiles = {}

            def kgrp(g):
                for mo, (o0, osz) in enumerate(CHUNKS):
                    kp = pp_pool.tile([128, 1024], F32, tag="pp")
                    for part, (cv, wv_) in enumerate(
                            ((c8m_v, wk8m_v), (c8t_v, wk8t_v))):
                        for i in range(2):
                            nt = 2 * g + i
                            nc.tensor.matmul(
                                kp[0:osz, i * 512:(i + 1) * 512],
                                wv_[:, :, o0:o0 + osz],
                                cv[:, :, nt * 512:(nt + 1) * 512],
                                start=(part == 0), stop=(part == 1),
                                perf_mode=DR,
                            )
                    for i in range(2):
                        nt = 2 * g + i
                        tmp = tmp_pool.tile([128, 512], BF16, tag="tmp")
                        qb = q_sb[0:osz,
                                  mo * 1024 + blk * 128:mo * 1024 + (blk + 1) * 128]
                        qb = qb.unsqueeze(1).to_broadcast((osz, 4, 128))
                        nc.vector.tensor_mul(
                            tmp[0:osz, :].rearrange("c (d p) -> c d p", d=4),
                            kp[0:osz, i * 512:(i + 1) * 512].rearrange(
                                "c (d p) -> c d p", d=4),
                            qb,
                        )
                        tmp_tiles[(nt, mo)] = tmp

            def selgrp(g):
                for i in range(2):
                    nt = 2 * g + i
                    sim = sp_pool.tile([8, 512], F32, tag="sp")
                    for mo, (o0, osz) in enumerate(CHUNKS):
                        nc.tensor.matmul(
                            sim[0:8, :],
                            sel_sb[0:osz, mo * 8:mo * 8 + 8],
                            tmp_tiles.pop((nt, mo))[0:osz, :],
                            start=(mo == 0), stop=(mo == 2),
                        )
                    nc.scalar.activation(s8[0:8, nt * 512:(nt + 1) * 512],
                                         sim[0:8, :],
                                         mybir.ActivationFunctionType.Exp)

            kgrp(0)
            kgrp(1)
            selgrp(0)
            kgrp(2)
            selgrp(1)
            kgrp(3)
            selgrp(2)
            selgrp(3)

            # broadcast attn rows 8 -> 320 via DRAM bounce on the scalar
            # HWDGE ring (sync ring carries the ctx loads)
            sc = s8d_t.ap()[blk % 2]
            nc.scalar.dma_start(sc, s8[0:8, :])
            ebc_sb = [ebc_pool.tile([128, DP], BF16, tag="ebc",
                                    name=f"ebc{mo}")
                      for mo in range(3)]
            for mo, (o0, osz) in enumerate(CHUNKS):
                nv = 64 if mo == 2 else VSZ[mo]
                for (r0, h0, nh, reps) in _bcast_runs(o0, nv):
                    src = sc[h0:h0 + nh, :].unsqueeze(1).to_broadcast(
                        (nh, reps, DP))
                    nc.scalar.dma_start(ebc_sb[mo][r0:r0 + nh * reps, :], src)
            nc.scalar.dma_start(ebc_sb[2][64:72, :], sc)  # raw attn for den
            return ctx_sb, ebc_sb

        def phase_b1(blk, ctx_sb, ebc_sb):
            """V projection (bf16, 2-nt PSUM groups), ACT drain, v*attn
            (DVE 2x), d-fold: step1 DVE, steps 2-5 GPSIMD."""
            mv_sb = [mv_pool.tile([128, DP], BF16, tag="mv", name=f"mv{mo}")
                     for mo in range(3)]
            for g in range(4):
                for mo, (o0, osz) in enumerate(CHUNKS):
                    vsz = VSZ[mo]
                    vp = pp_pool.tile([128, 1024], F32, tag="pp")
                    for kc, (c0, csz) in enumerate(CHUNKS):
                        ksz = csz + 1 if kc == 2 else csz
                        for i in range(2):
                            nt = 2 * g + i
                            nc.tensor.matmul(
                                vp[0:vsz, i * 512:(i + 1) * 512],
                                wv_sb[0:ksz, kc * 328 + o0:kc * 328 + o0 + vsz],
                                ctx_sb[kc][0:ksz, nt * 512:(nt + 1) * 512],
                                start=(kc == 0), stop=(kc == 2),
                            )
                    for i in range(2):
                        nt = 2 * g + i
                        vpd = vpd_pool.tile([128, 512], BF16, tag="vpd")
                        nc.scalar.activation(vpd[0:vsz, :],
                                             vp[0:vsz, i * 512:(i + 1) * 512],
                                             mybir.ActivationFunctionType.Copy)
                        nc.vector.tensor_mul(
                            mv_sb[mo][0:vsz, nt * 512:(nt + 1) * 512],
                            vpd[0:vsz, :],
                            ebc_sb[mo][0:vsz, nt * 512:(nt + 1) * 512],
                        )

            # d-reduce: fold 32 -> 1 over the outer free dim (d-major, so
            # every operand keeps a stride-1 innermost pix dim)
            ov_sb = ov_pool.tile([128, 384], F32, tag="ov")
            for mo in range(3):
                vsz = VSZ[mo]
                fs = fs_pool.tile([128, 3840], BF16, tag="fs")
                src = mv_sb[mo][0:vsz, :].rearrange("c (d p) -> c d p", d=D)
                o1 = fs[0:vsz, 0:2048].rearrange("c (d p) -> c d p", d=16)
                nc.vector.tensor_add(o1, src[:, 0:16, :], src[:, 16:32, :])
                cur = o1
                w = 16
                for off in (2048, 3072, 3584):
                    w //= 2
                    dst = fs[0:vsz, off:off + w * 128].rearrange(
                        "c (d p) -> c d p", d=w)
                    nc.gpsimd.tensor_add(dst, cur[:, 0:w, :], cur[:, w:2 * w, :])
                    cur = dst
                nc.gpsimd.tensor_add(
                    ov_sb[0:vsz, mo * 128:(mo + 1) * 128].rearrange(
                        "c (d p) -> c d p", d=1),
                    cur[:, 0:1, :], cur[:, 1:2, :])
            return ov_sb

        def phase_b2(blk, ov_sb):
            """Reciprocal, normalize, output projection, DMA out."""
            b = blk // (PIX_B // P)
            p0 = (blk % (PIX_B // P)) * P
            hr = p0 // W
            nh = P // W

            r8_sb = sm_pool.tile([128, P], F32R, tag="r8")
            with nc.allow_low_precision(reason="f32r reciprocal feeding matmul"):
                nc.vector.reciprocal(r8_sb[64:72, :], ov_sb[64:72, 2 * P:3 * P])
            att_sb = sm_pool.tile([128, 384], BF16, tag="att")
            r_ps = mp_pool.tile([128, 512], F32, tag="mp")
            for mo, (o0, osz) in enumerate(CHUNKS):
                nc.tensor.matmul(
                    r_ps[0:osz, mo * P:mo * P + P],
                    rsel_sb[64:72, mo * 128:mo * 128 + osz],
                    r8_sb[64:72, :],
                )
            # one normalize multiply over all three chunks; rows past each
            # chunk's VSZ are junk x junk and never read by the y-projection
            nc.vector.tensor_mul(
                att_sb[0:128, 0:384],
                ov_sb[0:128, 0:384],
                r_ps[0:128, 0:384],
            )

            y_ps = pp_pool.tile([128, 1024], F32, tag="pp")
            for mo, (o0, osz) in enumerate(CHUNKS):
                for kc, (c0, csz) in enumerate(CHUNKS):
                    nc.tensor.matmul(
                        y_ps[0:osz, mo * P:mo * P + P],
                        wo_sb[0:csz, kc * 320 + o0:kc * 320 + o0 + osz],
                        att_sb[0:csz, kc * P:kc * P + P],
                        start=(kc == 0), stop=(kc == 2),
                    )
            y_sb = y_pool.tile([128, 384], F32, tag="y")
            for mo, (o0, osz) in enumerate(CHUNKS):
                nc.scalar.add(
                    y_sb[0:osz, mo * P:mo * P + P],
                    y_ps[0:osz, mo * P:mo * P + P],
                    bout_sb[0:osz, mo:mo + 1],
                )
            for mo, (o0, osz) in enumerate(CHUNKS):
                dst = out_ap[b, o0:o0 + osz, hr:hr + nh, :].rearrange(
                    "c h w -> c (h w)")
                nc.scalar.dma_start(dst, y_sb[0:osz, mo * P:mo * P + P])

        # software pipeline, depth 2: A(s) | B2(s-1) | B1(s).  B1(s) runs
        # in the SAME step as A(s): the v-projection's ~19us of PE work
        # does not depend on the attn broadcast, so it covers the ~9us
        # bounce+bcast latency while keeping the PE dense (HAM warm) and
        # ctx prefetched exactly one block ahead.  B2 sits between so its
        # short DVE ops (recip/norm) queue ahead of B1's mul/fold chain.
        st_b = {}
        for s in range(NBLK + 1):
            if s < NBLK:
                ctx_sb, ebc_sb = phase_a(s)
            if s >= 1:
                phase_b2(s - 1, st_b.pop(s - 1))
            if s < NBLK:
                st_b[s] = phase_b1(s, ctx_sb, ebc_sb)

    nc.compile()
    return nc


_CACHED = {}


def _get_nc():
    if "nc" not in _CACHED:
        _CACHED["nc"] = build_nc()
    return _CACHED["nc"]


def make_core_inputs(x, context, wq, wk, wv, wout, bout):
    """Full inputs -> list of 8 per-core input dicts (host prep: shard,
    block, d-major layout, ones-channel, bf16 + fp8 DR panels)."""
    sel, rsel = make_constants()
    wk8m, wk8t = pack_wk_fp8(wk)
    consts = {
        "wq_p": pack_weight_T(np.asarray(wq, np.float32) * (SCALE / WK_SC)),
        "wk8m_p": wk8m, "wk8t_p": wk8t,
        "wv_p": pack_weight_T(np.asarray(wv, np.float32), ones_cols=True),
        "wo_p": pack_weight_T(np.asarray(wout, np.float32)),
        "sel_p": sel, "rsel_p": rsel,
    }
    bout_p = np.zeros((128, 3), np.float32)
    for mo, (o0, osz) in enumerate(CHUNKS):
        bout_p[0:osz, mo] = np.asarray(bout, np.float32)[o0:o0 + osz]
    consts["bout_p"] = bout_p
    x = np.asarray(x, np.float32)
    context = np.asarray(context, np.float32)
    nbh = PIX_B // P  # 4
    in_maps = []
    for cid in range(NCORES):
        h0 = cid * HLOC
        cs = context[:, :, :, h0:h0 + HLOC, :]  # [B, C, D, HLOC, W]
        # d-major: free = (d, pix), pix innermost
        cs = cs.reshape(B, CIN, D, nbh, P).transpose(0, 3, 1, 2, 4)
        cs = np.ascontiguousarray(cs.reshape(NBLK, CIN, DP), np.float32)
        panel = np.ones((NBLK, CIN + 1, DP), NPBF)
        panel[:, 0:CIN, :] = cs.astype(NPBF)
        # fp8 DoubleRow planar-pair panels: main c = j*128+p, tail c = 256+j*32+p
        c8 = to_fp8(cs)
        c8m = np.empty((NBLK, 128, 2 * DP), NPF8)
        c8m[:, :, 0:DP] = c8[:, 0:128, :]
        c8m[:, :, DP:2 * DP] = c8[:, 128:256, :]
        c8t = np.empty((NBLK, 32, 2 * DP), NPF8)
        c8t[:, :, 0:DP] = c8[:, 256:288, :]
        c8t[:, :, DP:2 * DP] = c8[:, 288:320, :]
        # x: [CIN, (blk, pix)]
        xs = x[:, :, h0:h0 + HLOC, :].reshape(B, CIN, nbh, P).transpose(1, 0, 2, 3)
        xs = np.ascontiguousarray(xs.reshape(CIN, NBLK * P), dtype=NPBF)
        m = dict(consts)
        m["ctx"] = panel
        m["c8m"] = c8m
        m["c8t"] = c8t
        m["x"] = xs
        in_maps.append(m)
    return in_maps


def kernel(x, context, wq, wk, wv, wout, bout):
    from concourse.bass_utils import run_bass_kernel_spmd

    nc = _get_nc()
    in_maps = make_core_inputs(x, context, wq, wk, wv, wout, bout)
    res = run_bass_kernel_spmd(nc, in_maps, list(range(NCORES)))
    shards = [res.results[c]["out"] for c in range(NCORES)]
    return np.concatenate(shards, axis=2).astype(np.float32)


if __name__ == "__main__":
    nc = build_nc()
    print("build + compile OK")


# revision 9
# speedup vs baseline: 1.0404x; 1.0404x over previous
"""Trainium2 Bass kernel for nn_DepthAttention (depth attention over d=32).

v2 design (from trace analysis of the 502us baseline):
  - PE was 89% occupied but at 35% MFU: every matmul paid a serialized
    ~107ns LDWEIGHTS, and the k-projection ran bf16.  Fixes:
      * k-projection in fp8 e4m3 DoubleRow (contraction 256+64 as two
        DR matmuls, 0.5 cyc/row) with host-packed planar-pair panels.
      * 2-nt-group PSUM tiles ([128,1024] = 2 banks) so each stationary
        serves 2 matmuls back-to-back.
  - d-major free layout (d outer, pix innermost) so the DVE broadcast
    operands have stride-1 innermost dims -> 2x_1P eligibility for the
    v*attn multiply and the fold tree.
  - Engine rebalance: DVE reads k-PSUM directly (fused drain+mul, 1x),
    ACT drains v (DVE v*attn then runs 2x from SBUF bf16), GPSIMD takes
    fold steps 2-5, exp stays on ACT.  Attn broadcast + outputs ride the
    scalar HWDGE ring; ctx loads ride the sync ring.
  - q-projection for all 8 blocks runs once upfront (also serves as the
    HAM warmup); wk is scaled x16 into fp8 range, compensated in wq.

Sharding: h (64) split across 8 cores -> 8 rows of h per core, no halo.
Per core: 1024 pixels in 8 blocks of P=128.  Softmax max-subtraction is
skipped (logits are O(1)).  The denominator comes free: a constant
ones-channel in the bf16 ctx panel makes the v-projection's chunk-2
matmul emit rows 64:72 = raw attn, whose d-fold is sum_d exp(sim).
"""

import sys

sys.path.insert(0, "/opt/trn_rl_repo")

from contextlib import ExitStack  # noqa: E402

import ml_dtypes  # noqa: E402
import numpy as np  # noqa: E402

import concourse.bacc as bacc  # noqa: E402
import concourse.bass as bass  # noqa: E402
import concourse.mybir as mybir  # noqa: E402
import concourse.tile as tile  # noqa: E402

HEADS = 8
DH = 40
CIN = 320
INNER = HEADS * DH  # 320
D = 32
B = 2
H = 64
W = 64
NCORES = 8
HLOC = H // NCORES  # 8
PIX_B = HLOC * W  # 512
P = 128
NBLK = B * PIX_B // P  # 8
DP = D * P  # 4096
NT = DP // 512  # 8
SCALE = DH ** -0.5
WK_SC = 16.0  # wk is scaled into fp8 range; compensated in wq

F32 = mybir.dt.float32
F32R = mybir.dt.float32r
BF16 = mybir.dt.bfloat16
FP8 = mybir.dt.float8e4
NPBF = ml_dtypes.bfloat16
NPF8 = ml_dtypes.float8_e4m3  # TRN FP8_EXP4: bias 7, max +-240
DR = mybir.MatmulPerfMode.DoubleRow

CHUNKS = [(0, 128), (128, 128), (256, 64)]
# v-projection output sizes: chunk2 carries 8 extra denominator rows
VSZ = [128, 128, 72]


def _head_of(c):
    return c // DH


def _bcast_runs(o0, nv):
    """Row-replication runs for broadcasting attn head-rows into a chunk's
    channel rows: list of (row0, head0, n_heads, reps_per_head)."""
    runs, r = [], 0
    while r < nv:
        c = o0 + r
        h = c // DH
        run = min((h + 1) * DH - c, nv - r)
        if run == DH:
            nh = 1
            while (r + (nh + 1) * DH <= nv and (o0 + r + nh * DH) % DH == 0):
                nh += 1
            runs.append((r, h, nh, DH))
            r += nh * DH
        else:
            runs.append((r, h, 1, run))
            r += run
    return runs


def make_constants():
    sel = np.zeros((128, 24), NPBF)
    for mo, (c0, csz) in enumerate(CHUNKS):
        for r in range(csz):
            sel[r, mo * 8 + _head_of(c0 + r)] = 1.0
    rsel = np.zeros((8, 384), np.float32)
    for mo, (c0, csz) in enumerate(CHUNKS):
        for r in range(csz):
            rsel[_head_of(c0 + r), mo * 128 + r] = 1.0
    return sel, rsel


def to_fp8(x):
    return np.clip(x, -240.0, 240.0).astype(NPF8)


def pack_weight_T(w, ones_cols=False):
    """w [out, in] -> bf16 packed lhsT [128, 3*M] with M = out (+8 den
    cols when ones_cols).  Chunk kc of the 'in' dim at free offset kc*M;
    chunk 2 gets an extra contraction row 64 (the ones-channel), wired to
    the 8 denominator columns when ones_cols."""
    wt = np.ascontiguousarray(w.T, dtype=np.float32)  # [in, out]
    od = wt.shape[1]
    m = od + 8 if ones_cols else od
    p = np.zeros((128, 3 * m), NPBF)
    for kc, (c0, csz) in enumerate(CHUNKS):
        p[0:csz, kc * m:kc * m + od] = wt[c0:c0 + csz, :]
    if ones_cols:
        for n in range(8):
            p[64, 2 * m + od + n] = 1.0  # ones-channel -> den col n (chunk2)
    return p


def pack_wk_fp8(wk):
    """wk [320, 320] -> DoubleRow planar-pair stationaries.
    main [128, 2*320]: [p, j*320+m] = wk_sc[m, j*128+p]  (c = j*128+p)
    tail [32, 2*320]:  [p, j*320+m] = wk_sc[m, 256+j*32+p]"""
    wk_sc = np.asarray(wk, np.float32) * WK_SC
    main = np.zeros((128, 640), NPF8)
    tailp = np.zeros((32, 640), NPF8)
    for j in range(2):
        main[:, j * 320:(j + 1) * 320] = to_fp8(wk_sc[:, j * 128:(j + 1) * 128].T)
        tailp[:, j * 320:(j + 1) * 320] = to_fp8(
            wk_sc[:, 256 + j * 32:256 + (j + 1) * 32].T)
    return main, tailp


def build_nc():
    nc = bacc.Bacc(
        "TRN2",
        target_bir_lowering=False,
        debug=False,
        enable_asserts=False,
        num_devices=NCORES,
    )

    ctx_t = nc.dram_tensor("ctx", [NBLK, CIN + 1, DP], BF16, kind="ExternalInput")
    c8m_t = nc.dram_tensor("c8m", [NBLK, 128, 2 * DP], FP8, kind="ExternalInput")
    c8t_t = nc.dram_tensor("c8t", [NBLK, 32, 2 * DP], FP8, kind="ExternalInput")
    s8d_t = nc.dram_tensor("s8d", [2, 8, DP], BF16, kind="Internal")
    x_t = nc.dram_tensor("x", [CIN, NBLK * P], BF16, kind="ExternalInput")
    wq_t = nc.dram_tensor("wq_p", [128, 960], BF16, kind="ExternalInput")
    wk8m_t = nc.dram_tensor("wk8m_p", [128, 640], FP8, kind="ExternalInput")
    wk8t_t = nc.dram_tensor("wk8t_p", [32, 640], FP8, kind="ExternalInput")
    wv_t = nc.dram_tensor("wv_p", [128, 984], BF16, kind="ExternalInput")
    wo_t = nc.dram_tensor("wo_p", [128, 960], BF16, kind="ExternalInput")
    sel_t = nc.dram_tensor("sel_p", [128, 24], BF16, kind="ExternalInput")
    rsel_t = nc.dram_tensor("rsel_p", [8, 384], F32R, kind="ExternalInput")
    bout_t = nc.dram_tensor("bout_p", [128, 3], F32, kind="ExternalInput")
    out_t = nc.dram_tensor("out", [B, INNER, HLOC, W], F32, kind="ExternalOutput")

    ctx_ap = ctx_t.ap()
    out_ap = out_t.ap()

    with tile.TileContext(nc) as tc, ExitStack() as ctxs:
        ep = ctxs.enter_context

        const_pool = ep(tc.tile_pool(name="const", bufs=1))
        qx_pool = ep(tc.tile_pool(name="qx", bufs=1))
        ctx_pool = ep(tc.tile_pool(name="ctxp", bufs=6))
        c8m_pool = ep(tc.tile_pool(name="c8mp", bufs=2))
        c8t_pool = ep(tc.tile_pool(name="c8tp", bufs=2))
        tmp_pool = ep(tc.tile_pool(name="tmpp", bufs=6))
        s8_pool = ep(tc.tile_pool(name="s8p", bufs=2))
        ebc_pool = ep(tc.tile_pool(name="ebcp", bufs=4))
        vpd_pool = ep(tc.tile_pool(name="vpdp", bufs=4))
        mv_pool = ep(tc.tile_pool(name="mvp", bufs=3))
        fs_pool = ep(tc.tile_pool(name="fsp", bufs=2))
        ov_pool = ep(tc.tile_pool(name="ovp", bufs=2))
        sm_pool = ep(tc.tile_pool(name="smp", bufs=2))
        y_pool = ep(tc.tile_pool(name="yp", bufs=2))

        pp_pool = ep(tc.tile_pool(name="pp", bufs=3, space="PSUM"))
        sp_pool = ep(tc.tile_pool(name="sp", bufs=1, space="PSUM"))
        mp_pool = ep(tc.tile_pool(name="mp", bufs=1, space="PSUM"))

        # ---- constants ----
        wk8m_sb = const_pool.tile([128, 640], FP8, tag="wk8m")
        wk8t_sb = const_pool.tile([32, 640], FP8, tag="wk8t")
        wq_sb = const_pool.tile([128, 960], BF16, tag="wq")
        wv_sb = const_pool.tile([128, 984], BF16, tag="wv")
        wo_sb = const_pool.tile([128, 960], BF16, tag="wo")
        sel_sb = const_pool.tile([128, 24], BF16, tag="sel")
        rsel_sb = const_pool.tile([128, 384], F32R, tag="rsel")
        bout_sb = const_pool.tile([128, 3], F32, tag="bout")
        x_sb = qx_pool.tile([128, 3 * NBLK * P], BF16, tag="xall")
        q_sb = qx_pool.tile([128, 3 * NBLK * P], BF16, tag="qall")

        for kc, (c0, csz) in enumerate(CHUNKS):
            nc.sync.dma_start(x_sb[0:csz, kc * 1024:(kc + 1) * 1024],
                              x_t.ap()[c0:c0 + csz, :])
        nc.sync.dma_start(wq_sb[:], wq_t.ap())
        nc.sync.dma_start(wk8m_sb[:], wk8m_t.ap())
        nc.sync.dma_start(wk8t_sb[:], wk8t_t.ap())
        nc.sync.dma_start(wv_sb[:], wv_t.ap())
        nc.sync.dma_start(wo_sb[:], wo_t.ap())
        nc.sync.dma_start(sel_sb[:], sel_t.ap())
        nc.sync.dma_start(rsel_sb[64:72, :], rsel_t.ap())
        nc.sync.dma_start(bout_sb[:], bout_t.ap())

        wk8m_v = wk8m_sb[:].rearrange("p (j m) -> p j m", j=2)
        wk8t_v = wk8t_sb[:].rearrange("p (j m) -> p j m", j=2)

        # ---- q projection for all 8 blocks (also HAM warmup) ----
        # q layout: q_sb [128, (mo, blk, pix)]
        for mo, (o0, osz) in enumerate(CHUNKS):
            qp = pp_pool.tile([128, 1024], F32, tag="pp")
            for kc, (c0, csz) in enumerate(CHUNKS):
                for half in range(2):
                    nc.tensor.matmul(
                        qp[0:osz, half * 512:(half + 1) * 512],
                        wq_sb[0:csz, kc * 320 + o0:kc * 320 + o0 + osz],
                        x_sb[0:csz, kc * 1024 + half * 512:kc * 1024 + (half + 1) * 512],
                        start=(kc == 0), stop=(kc == 2),
                    )
            nc.scalar.activation(q_sb[0:osz, mo * 1024:(mo + 1) * 1024],
                                 qp[0:osz, :],
                                 mybir.ActivationFunctionType.Copy)

        def phase_a(blk):
            """ctx DMA, fp8-DR k proj, k*q (DVE from PSUM), selector
            reduce, exp, attn broadcast via DRAM bounce."""
            c8m = c8m_pool.tile([128, 2 * DP], FP8, tag="c8m")
            nc.sync.dma_start(c8m[:], c8m_t.ap()[blk])
            c8t = c8t_pool.tile([32, 2 * DP], FP8, tag="c8t")
            nc.sync.dma_start(c8t[:], c8t_t.ap()[blk])
            ctx_sb = []
            for kc, (c0, csz) in enumerate(CHUNKS):
                t = ctx_pool.tile([128, DP], BF16, tag="ctx")
                ksz = csz + 1 if kc == 2 else csz  # chunk2 + ones-channel
                nc.sync.dma_start(t[0:ksz, :], ctx_ap[blk, c0:c0 + ksz, :])
                ctx_sb.append(t)

            c8m_v = c8m[:].rearrange("p (j f) -> p j f", j=2)
            c8t_v = c8t[:].rearrange("p (j f) -> p j f", j=2)
            s8 = s8_pool.tile([8, DP], BF16, tag="s8")
            tmp_tiles = {}

            def kgrp(g):
                for mo, (o0, osz) in enumerate(CHUNKS):
                    kp = pp_pool.tile([128, 1024], F32, tag="pp")
                    for part, (cv, wv_) in enumerate(
                            ((c8m_v, wk8m_v), (c8t_v, wk8t_v))):
                        for i in range(2):
                            nt = 2 * g + i
                            nc.tensor.matmul(
                                kp[0:osz, i * 512:(i + 1) * 512],
                                wv_[:, :, o0:o0 + osz],
                                cv[:, :, nt * 512:(nt + 1) * 512],
                                start=(part == 0), stop=(part == 1),
                                perf_mode=DR,
                            )
                    for i in range(2):
                        nt = 2 * g + i
                        tmp = tmp_pool.tile([128, 512], BF16, tag="tmp")
                        qb = q_sb[0:osz,
                                  mo * 1024 + blk * 128:mo * 1024 + (blk + 1) * 128]
                        qb = qb.unsqueeze(1).to_broadcast((osz, 4, 128))
                        nc.vector.tensor_mul(
                            tmp[0:osz, :].rearrange("c (d p) -> c d p", d=4),
                            kp[0:osz, i * 512:(i + 1) * 512].rearrange(
                                "c (d p) -> c d p", d=4),
                            qb,
                        )
                        tmp_tiles[(nt, mo)] = tmp

            def selgrp(g):
                for i in range(2):
                    nt = 2 * g + i
                    sim = sp_pool.tile([8, 512], F32, tag="sp")
                    for mo, (o0, osz) in enumerate(CHUNKS):
                        nc.tensor.matmul(
                            sim[0:8, :],
                            sel_sb[0:osz, mo * 8:mo * 8 + 8],
                            tmp_tiles.pop((nt, mo))[0:osz, :],
                            start=(mo == 0), stop=(mo == 2),
                        )
                    nc.scalar.activation(s8[0:8, nt * 512:(nt + 1) * 512],
                                         sim[0:8, :],
                                         mybir.ActivationFunctionType.Exp)

            kgrp(0)
            kgrp(1)
            selgrp(0)
            kgrp(2)
            selgrp(1)
            kgrp(3)
            selgrp(2)
            selgrp(3)

            # broadcast attn rows 8 -> 320 via DRAM bounce.  Issued from
            # the GpSimd queue: these DMAs are gated on compute (s8) and
            # would head-of-line-block the ctx loads (sync ring) or the
            # vpd drains (scalar engine) if issued there.
            sc = s8d_t.ap()[blk % 2]
            nc.gpsimd.dma_start(sc, s8[0:8, :])
            ebc_sb = [ebc_pool.tile([128, DP], BF16, tag="ebc",
                                    name=f"ebc{mo}")
                      for mo in range(3)]
            for mo, (o0, osz) in enumerate(CHUNKS):
                nv = 64 if mo == 2 else VSZ[mo]
                for (r0, h0, nh, reps) in _bcast_runs(o0, nv):
                    src = sc[h0:h0 + nh, :].unsqueeze(1).to_broadcast(
                        (nh, reps, DP))
                    nc.gpsimd.dma_start(ebc_sb[mo][r0:r0 + nh * reps, :], src)
            nc.gpsimd.dma_start(ebc_sb[2][64:72, :], sc)  # raw attn for den
            return ctx_sb, ebc_sb

        def phase_b1(blk, ctx_sb, ebc_sb):
            """V projection (bf16, 2-nt PSUM groups), ACT drain, v*attn
            (DVE 2x), d-fold: step1 DVE, steps 2-5 GPSIMD."""
            mv_sb = [mv_pool.tile([128, DP], BF16, tag="mv", name=f"mv{mo}")
                     for mo in range(3)]
            for g in range(4):
                for mo, (o0, osz) in enumerate(CHUNKS):
                    vsz = VSZ[mo]
                    vp = pp_pool.tile([128, 1024], F32, tag="pp")
                    for kc, (c0, csz) in enumerate(CHUNKS):
                        ksz = csz + 1 if kc == 2 else csz
                        for i in range(2):
                            nt = 2 * g + i
                            nc.tensor.matmul(
                                vp[0:vsz, i * 512:(i + 1) * 512],
                                wv_sb[0:ksz, kc * 328 + o0:kc * 328 + o0 + vsz],
                                ctx_sb[kc][0:ksz, nt * 512:(nt + 1) * 512],
                                start=(kc == 0), stop=(kc == 2),
                            )
                    for i in range(2):
                        nt = 2 * g + i
                        vpd = vpd_pool.tile([128, 512], BF16, tag="vpd")
                        nc.scalar.activation(vpd[0:vsz, :],
                                             vp[0:vsz, i * 512:(i + 1) * 512],
                                             mybir.ActivationFunctionType.Copy)
                        nc.vector.tensor_mul(
                            mv_sb[mo][0:vsz, nt * 512:(nt + 1) * 512],
                            vpd[0:vsz, :],
                            ebc_sb[mo][0:vsz, nt * 512:(nt + 1) * 512],
                        )

            # d-reduce: fold 32 -> 1 over the outer free dim (d-major, so
            # every operand keeps a stride-1 innermost pix dim)
            ov_sb = ov_pool.tile([128, 384], F32, tag="ov")
            for mo in range(3):
                vsz = VSZ[mo]
                fs = fs_pool.tile([128, 3840], BF16, tag="fs")
                src = mv_sb[mo][0:vsz, :].rearrange("c (d p) -> c d p", d=D)
                o1 = fs[0:vsz, 0:2048].rearrange("c (d p) -> c d p", d=16)
                nc.vector.tensor_add(o1, src[:, 0:16, :], src[:, 16:32, :])
                cur = o1
                w = 16
                for off in (2048, 3072, 3584):
                    w //= 2
                    dst = fs[0:vsz, off:off + w * 128].rearrange(
                        "c (d p) -> c d p", d=w)
                    nc.vector.tensor_add(dst, cur[:, 0:w, :], cur[:, w:2 * w, :])
                    cur = dst
                nc.vector.tensor_add(
                    ov_sb[0:vsz, mo * 128:(mo + 1) * 128].rearrange(
                        "c (d p) -> c d p", d=1),
                    cur[:, 0:1, :], cur[:, 1:2, :])
            return ov_sb

        def phase_b2(blk, ov_sb):
            """Reciprocal, normalize, output projection, DMA out."""
            b = blk // (PIX_B // P)
            p0 = (blk % (PIX_B // P)) * P
            hr = p0 // W
            nh = P // W

            r8_sb = sm_pool.tile([128, P], F32R, tag="r8")
            with nc.allow_low_precision(reason="f32r reciprocal feeding matmul"):
                nc.vector.reciprocal(r8_sb[64:72, :], ov_sb[64:72, 2 * P:3 * P])
            att_sb = sm_pool.tile([128, 384], BF16, tag="att")
            r_ps = mp_pool.tile([128, 512], F32, tag="mp")
            for mo, (o0, osz) in enumerate(CHUNKS):
                nc.tensor.matmul(
                    r_ps[0:osz, mo * P:mo * P + P],
                    rsel_sb[64:72, mo * 128:mo * 128 + osz],
                    r8_sb[64:72, :],
                )
            # one normalize multiply over all three chunks; rows past each
            # chunk's VSZ are junk x junk and never read by the y-projection
            nc.vector.tensor_mul(
                att_sb[0:128, 0:384],
                ov_sb[0:128, 0:384],
                r_ps[0:128, 0:384],
            )

            y_ps = pp_pool.tile([128, 1024], F32, tag="pp")
            for mo, (o0, osz) in enumerate(CHUNKS):
                for kc, (c0, csz) in enumerate(CHUNKS):
                    nc.tensor.matmul(
                        y_ps[0:osz, mo * P:mo * P + P],
                        wo_sb[0:csz, kc * 320 + o0:kc * 320 + o0 + osz],
                        att_sb[0:csz, kc * P:kc * P + P],
                        start=(kc == 0), stop=(kc == 2),
                    )
            y_sb = y_pool.tile([128, 384], F32, tag="y")
            for mo, (o0, osz) in enumerate(CHUNKS):
                nc.scalar.add(
                    y_sb[0:osz, mo * P:mo * P + P],
                    y_ps[0:osz, mo * P:mo * P + P],
                    bout_sb[0:osz, mo:mo + 1],
                )
            for mo, (o0, osz) in enumerate(CHUNKS):
                dst = out_ap[b, o0:o0 + osz, hr:hr + nh, :].rearrange(
                    "c h w -> c (h w)")
                nc.scalar.dma_start(dst, y_sb[0:osz, mo * P:mo * P + P])

        # software pipeline, depth 2: A(s) | B2(s-1) | B1(s).  B1(s) runs
        # in the SAME step as A(s): the v-projection's ~19us of PE work
        # does not depend on the attn broadcast, so it covers the ~9us
        # bounce+bcast latency while keeping the PE dense (HAM warm) and
        # ctx prefetched exactly one block ahead.  B2 sits between so its
        # short DVE ops (recip/norm) queue ahead of B1's mul/fold chain.
        st_b = {}
        for s in range(NBLK + 1):
            if s < NBLK:
                ctx_sb, ebc_sb = phase_a(s)
            if s >= 1:
                phase_b2(s - 1, st_b.pop(s - 1))
            if s < NBLK:
                st_b[s] = phase_b1(s, ctx_sb, ebc_sb)

    nc.compile()
    return nc


_CACHED = {}


def _get_nc():
    if "nc" not in _CACHED:
        _CACHED["nc"] = build_nc()
    return _CACHED["nc"]


def make_core_inputs(x, context, wq, wk, wv, wout, bout):
    """Full inputs -> list of 8 per-core input dicts (host prep: shard,
    block, d-major layout, ones-channel, bf16 + fp8 DR panels)."""
    sel, rsel = make_constants()
    wk8m, wk8t = pack_wk_fp8(wk)
    consts = {
        "wq_p": pack_weight_T(np.asarray(wq, np.float32) * (SCALE / WK_SC)),
        "wk8m_p": wk8m, "wk8t_p": wk8t,
        "wv_p": pack_weight_T(np.asarray(wv, np.float32), ones_cols=True),
        "wo_p": pack_weight_T(np.asarray(wout, np.float32)),
        "sel_p": sel, "rsel_p": rsel,
    }
    bout_p = np.zeros((128, 3), np.float32)
    for mo, (o0, osz) in enumerate(CHUNKS):
        bout_p[0:osz, mo] = np.asarray(bout, np.float32)[o0:o0 + osz]
    consts["bout_p"] = bout_p
    x = np.asarray(x, np.float32)
    context = np.asarray(context, np.float32)
    nbh = PIX_B // P  # 4
    in_maps = []
    for cid in range(NCORES):
        h0 = cid * HLOC
        cs = context[:, :, :, h0:h0 + HLOC, :]  # [B, C, D, HLOC, W]
        # d-major: free = (d, pix), pix innermost
        cs = cs.reshape(B, CIN, D, nbh, P).transpose(0, 3, 1, 2, 4)
        cs = np.ascontiguousarray(cs.reshape(NBLK, CIN, DP), np.float32)
        panel = np.ones((NBLK, CIN + 1, DP), NPBF)
        panel[:, 0:CIN, :] = cs.astype(NPBF)
        # fp8 DoubleRow planar-pair panels: main c = j*128+p, tail c = 256+j*32+p
        c8 = to_fp8(cs)
        c8m = np.empty((NBLK, 128, 2 * DP), NPF8)
        c8m[:, :, 0:DP] = c8[:, 0:128, :]
        c8m[:, :, DP:2 * DP] = c8[:, 128:256, :]
        c8t = np.empty((NBLK, 32, 2 * DP), NPF8)
        c8t[:, :, 0:DP] = c8[:, 256:288, :]
        c8t[:, :, DP:2 * DP] = c8[:, 288:320, :]
        # x: [CIN, (blk, pix)]
        xs = x[:, :, h0:h0 + HLOC, :].reshape(B, CIN, nbh, P).transpose(1, 0, 2, 3)
        xs = np.ascontiguousarray(xs.reshape(CIN, NBLK * P), dtype=NPBF)
        m = dict(consts)
        m["ctx"] = panel
        m["c8m"] = c8m
        m["c8t"] = c8t
        m["x"] = xs
        in_maps.append(m)
    return in_maps


def kernel(x, context, wq, wk, wv, wout, bout):
    from concourse.bass_utils import run_bass_kernel_spmd

    nc = _get_nc()
    in_maps = make_core_inputs(x, context, wq, wk, wv, wout, bout)
    res = run_bass_kernel_spmd(nc, in_maps, list(range(NCORES)))
    shards = [res.results[c]["out"] for c in range(NCORES)]
    return np.concatenate(shards, axis=2).astype(np.float32)


if __name__ == "__main__":
    nc = build_nc()
    print("build + compile OK")


# revision 12
# speedup vs baseline: 1.0566x; 1.0155x over previous
"""Trainium2 Bass kernel for nn_DepthAttention (depth attention over d=32).

v2 design (from trace analysis of the 502us baseline):
  - PE was 89% occupied but at 35% MFU: every matmul paid a serialized
    ~107ns LDWEIGHTS, and the k-projection ran bf16.  Fixes:
      * k-projection in fp8 e4m3 DoubleRow (contraction 256+64 as two
        DR matmuls, 0.5 cyc/row) with host-packed planar-pair panels.
      * 2-nt-group PSUM tiles ([128,1024] = 2 banks) so each stationary
        serves 2 matmuls back-to-back.
  - d-major free layout (d outer, pix innermost) so the DVE broadcast
    operands have stride-1 innermost dims -> 2x_1P eligibility for the
    v*attn multiply and the fold tree.
  - Engine rebalance: DVE reads k-PSUM directly (fused drain+mul, 1x),
    ACT drains v (DVE v*attn then runs 2x from SBUF bf16), GPSIMD takes
    fold steps 2-5, exp stays on ACT.  Attn broadcast + outputs ride the
    scalar HWDGE ring; ctx loads ride the sync ring.
  - q-projection for all 8 blocks runs once upfront (also serves as the
    HAM warmup); wk is scaled x16 into fp8 range, compensated in wq.

Sharding: h (64) split across 8 cores -> 8 rows of h per core, no halo.
Per core: 1024 pixels in 8 blocks of P=128.  Softmax max-subtraction is
skipped (logits are O(1)).  The denominator comes free: a constant
ones-channel in the bf16 ctx panel makes the v-projection's chunk-2
matmul emit rows 64:72 = raw attn, whose d-fold is sum_d exp(sim).
"""

import sys

sys.path.insert(0, "/opt/trn_rl_repo")

from contextlib import ExitStack  # noqa: E402

import ml_dtypes  # noqa: E402
import numpy as np  # noqa: E402

import concourse.bacc as bacc  # noqa: E402
import concourse.bass as bass  # noqa: E402
import concourse.mybir as mybir  # noqa: E402
import concourse.tile as tile  # noqa: E402

HEADS = 8
DH = 40
CIN = 320
INNER = HEADS * DH  # 320
D = 32
B = 2
H = 64
W = 64
NCORES = 8
HLOC = H // NCORES  # 8
PIX_B = HLOC * W  # 512
P = 128
NBLK = B * PIX_B // P  # 8
DP = D * P  # 4096
NT = DP // 512  # 8
SCALE = DH ** -0.5
WK_SC = 16.0  # wk is scaled into fp8 range; compensated in wq

F32 = mybir.dt.float32
F32R = mybir.dt.float32r
BF16 = mybir.dt.bfloat16
FP8 = mybir.dt.float8e4
NPBF = ml_dtypes.bfloat16
NPF8 = ml_dtypes.float8_e4m3  # TRN FP8_EXP4: bias 7, max +-240
DR = mybir.MatmulPerfMode.DoubleRow

CHUNKS = [(0, 128), (128, 128), (256, 64)]
# v-projection output sizes: chunk2 carries 8 extra denominator rows
VSZ = [128, 128, 72]


def _head_of(c):
    return c // DH


def _bcast_runs(o0, nv):
    """Row-replication runs for broadcasting attn head-rows into a chunk's
    channel rows: list of (row0, head0, n_heads, reps_per_head)."""
    runs, r = [], 0
    while r < nv:
        c = o0 + r
        h = c // DH
        run = min((h + 1) * DH - c, nv - r)
        if run == DH:
            nh = 1
            while (r + (nh + 1) * DH <= nv and (o0 + r + nh * DH) % DH == 0):
                nh += 1
            runs.append((r, h, nh, DH))
            r += nh * DH
        else:
            runs.append((r, h, 1, run))
            r += run
    return runs


def make_constants():
    sel = np.zeros((128, 24), NPBF)
    for mo, (c0, csz) in enumerate(CHUNKS):
        for r in range(csz):
            sel[r, mo * 8 + _head_of(c0 + r)] = 1.0
    rsel = np.zeros((8, 384), np.float32)
    for mo, (c0, csz) in enumerate(CHUNKS):
        for r in range(csz):
            rsel[_head_of(c0 + r), mo * 128 + r] = 1.0
    return sel, rsel


def to_fp8(x):
    return np.clip(x, -240.0, 240.0).astype(NPF8)


def pack_weight_T(w, ones_cols=False):
    """w [out, in] -> bf16 packed lhsT [128, 3*M] with M = out (+8 den
    cols when ones_cols).  Chunk kc of the 'in' dim at free offset kc*M;
    chunk 2 gets an extra contraction row 64 (the ones-channel), wired to
    the 8 denominator columns when ones_cols."""
    wt = np.ascontiguousarray(w.T, dtype=np.float32)  # [in, out]
    od = wt.shape[1]
    m = od + 8 if ones_cols else od
    p = np.zeros((128, 3 * m), NPBF)
    for kc, (c0, csz) in enumerate(CHUNKS):
        p[0:csz, kc * m:kc * m + od] = wt[c0:c0 + csz, :]
    if ones_cols:
        for n in range(8):
            p[64, 2 * m + od + n] = 1.0  # ones-channel -> den col n (chunk2)
    return p


def pack_wk_fp8(wk):
    """wk [320, 320] -> DoubleRow planar-pair stationaries.
    main [128, 2*320]: [p, j*320+m] = wk_sc[m, j*128+p]  (c = j*128+p)
    tail [32, 2*320]:  [p, j*320+m] = wk_sc[m, 256+j*32+p]"""
    wk_sc = np.asarray(wk, np.float32) * WK_SC
    main = np.zeros((128, 640), NPF8)
    tailp = np.zeros((32, 640), NPF8)
    for j in range(2):
        main[:, j * 320:(j + 1) * 320] = to_fp8(wk_sc[:, j * 128:(j + 1) * 128].T)
        tailp[:, j * 320:(j + 1) * 320] = to_fp8(
            wk_sc[:, 256 + j * 32:256 + (j + 1) * 32].T)
    return main, tailp


def build_nc():
    nc = bacc.Bacc(
        "TRN2",
        target_bir_lowering=False,
        debug=False,
        enable_asserts=False,
        num_devices=NCORES,
    )

    ctx_t = nc.dram_tensor("ctx", [NBLK, CIN + 1, DP], BF16, kind="ExternalInput")
    c8m_t = nc.dram_tensor("c8m", [NBLK, 128, 2 * DP], FP8, kind="ExternalInput")
    c8t_t = nc.dram_tensor("c8t", [NBLK, 32, 2 * DP], FP8, kind="ExternalInput")
    s8d_t = nc.dram_tensor("s8d", [2, 8, DP], BF16, kind="Internal")
    x_t = nc.dram_tensor("x", [CIN, NBLK * P], BF16, kind="ExternalInput")
    wq_t = nc.dram_tensor("wq_p", [128, 960], BF16, kind="ExternalInput")
    wk8m_t = nc.dram_tensor("wk8m_p", [128, 640], FP8, kind="ExternalInput")
    wk8t_t = nc.dram_tensor("wk8t_p", [32, 640], FP8, kind="ExternalInput")
    wv_t = nc.dram_tensor("wv_p", [128, 984], BF16, kind="ExternalInput")
    wo_t = nc.dram_tensor("wo_p", [128, 960], BF16, kind="ExternalInput")
    sel_t = nc.dram_tensor("sel_p", [128, 24], BF16, kind="ExternalInput")
    rsel_t = nc.dram_tensor("rsel_p", [8, 384], F32R, kind="ExternalInput")
    bout_t = nc.dram_tensor("bout_p", [128, 3], F32, kind="ExternalInput")
    out_t = nc.dram_tensor("out", [B, INNER, HLOC, W], F32, kind="ExternalOutput")

    ctx_ap = ctx_t.ap()
    out_ap = out_t.ap()

    with tile.TileContext(nc) as tc, ExitStack() as ctxs:
        ep = ctxs.enter_context

        const_pool = ep(tc.tile_pool(name="const", bufs=1))
        qx_pool = ep(tc.tile_pool(name="qx", bufs=1))
        ctx_pool = ep(tc.tile_pool(name="ctxp", bufs=6))
        c8m_pool = ep(tc.tile_pool(name="c8mp", bufs=2))
        c8t_pool = ep(tc.tile_pool(name="c8tp", bufs=2))
        tmp_pool = ep(tc.tile_pool(name="tmpp", bufs=6))
        s8_pool = ep(tc.tile_pool(name="s8p", bufs=2))
        ebc_pool = ep(tc.tile_pool(name="ebcp", bufs=4))
        vpd_pool = ep(tc.tile_pool(name="vpdp", bufs=4))
        mv_pool = ep(tc.tile_pool(name="mvp", bufs=3))
        fs_pool = ep(tc.tile_pool(name="fsp", bufs=2))
        ov_pool = ep(tc.tile_pool(name="ovp", bufs=2))
        sm_pool = ep(tc.tile_pool(name="smp", bufs=2))
        y_pool = ep(tc.tile_pool(name="yp", bufs=2))

        pp_pool = ep(tc.tile_pool(name="pp", bufs=3, space="PSUM"))
        sp_pool = ep(tc.tile_pool(name="sp", bufs=1, space="PSUM"))
        mp_pool = ep(tc.tile_pool(name="mp", bufs=1, space="PSUM"))

        # ---- constants ----
        wk8m_sb = const_pool.tile([128, 640], FP8, tag="wk8m")
        wk8t_sb = const_pool.tile([32, 640], FP8, tag="wk8t")
        wq_sb = const_pool.tile([128, 960], BF16, tag="wq")
        wv_sb = const_pool.tile([128, 984], BF16, tag="wv")
        wo_sb = const_pool.tile([128, 960], BF16, tag="wo")
        sel_sb = const_pool.tile([128, 24], BF16, tag="sel")
        rsel_sb = const_pool.tile([128, 384], F32R, tag="rsel")
        bout_sb = const_pool.tile([128, 3], F32, tag="bout")
        x_sb = qx_pool.tile([128, 3 * NBLK * P], BF16, tag="xall")
        q_sb = qx_pool.tile([128, 3 * NBLK * P], BF16, tag="qall")

        for kc, (c0, csz) in enumerate(CHUNKS):
            nc.sync.dma_start(x_sb[0:csz, kc * 1024:(kc + 1) * 1024],
                              x_t.ap()[c0:c0 + csz, :])
        nc.sync.dma_start(wq_sb[:], wq_t.ap())
        nc.sync.dma_start(wk8m_sb[:], wk8m_t.ap())
        nc.sync.dma_start(wk8t_sb[:], wk8t_t.ap())
        nc.sync.dma_start(wv_sb[:], wv_t.ap())
        nc.sync.dma_start(wo_sb[:], wo_t.ap())
        nc.sync.dma_start(sel_sb[:], sel_t.ap())
        nc.sync.dma_start(rsel_sb[64:72, :], rsel_t.ap())
        nc.sync.dma_start(bout_sb[:], bout_t.ap())

        wk8m_v = wk8m_sb[:].rearrange("p (j m) -> p j m", j=2)
        wk8t_v = wk8t_sb[:].rearrange("p (j m) -> p j m", j=2)

        # ---- q projection for all 8 blocks (also HAM warmup) ----
        # q layout: q_sb [128, (mo, blk, pix)]
        for mo, (o0, osz) in enumerate(CHUNKS):
            qp = pp_pool.tile([128, 1024], F32, tag="pp")
            for kc, (c0, csz) in enumerate(CHUNKS):
                for half in range(2):
                    nc.tensor.matmul(
                        qp[0:osz, half * 512:(half + 1) * 512],
                        wq_sb[0:csz, kc * 320 + o0:kc * 320 + o0 + osz],
                        x_sb[0:csz, kc * 1024 + half * 512:kc * 1024 + (half + 1) * 512],
                        start=(kc == 0), stop=(kc == 2),
                    )
            nc.scalar.activation(q_sb[0:osz, mo * 1024:(mo + 1) * 1024],
                                 qp[0:osz, :],
                                 mybir.ActivationFunctionType.Copy)

        def phase_a(blk):
            """ctx DMA, fp8-DR k proj, k*q (DVE from PSUM), selector
            reduce, exp, attn broadcast via DRAM bounce."""
            c8m = c8m_pool.tile([128, 2 * DP], FP8, tag="c8m")
            nc.sync.dma_start(c8m[:], c8m_t.ap()[blk])
            c8t = c8t_pool.tile([32, 2 * DP], FP8, tag="c8t")
            nc.sync.dma_start(c8t[:], c8t_t.ap()[blk])
            # ctx bf16 rides the scalar ring: on the sync ring it would sit
            # behind the compute-gated bounce/bcast triggers; on scalar only
            # the fast-firing y-out triggers precede it
            ctx_sb = []
            for kc, (c0, csz) in enumerate(CHUNKS):
                t = ctx_pool.tile([128, DP], BF16, tag="ctx")
                ksz = csz + 1 if kc == 2 else csz  # chunk2 + ones-channel
                nc.scalar.dma_start(t[0:ksz, :], ctx_ap[blk, c0:c0 + ksz, :])
                ctx_sb.append(t)

            c8m_v = c8m[:].rearrange("p (j f) -> p j f", j=2)
            c8t_v = c8t[:].rearrange("p (j f) -> p j f", j=2)
            s8 = s8_pool.tile([8, DP], BF16, tag="s8")
            tmp_tiles = {}

            def kgrp(g):
                for mo, (o0, osz) in enumerate(CHUNKS):
                    kp = pp_pool.tile([128, 1024], F32, tag="pp")
                    for part, (cv, wv_) in enumerate(
                            ((c8m_v, wk8m_v), (c8t_v, wk8t_v))):
                        for i in range(2):
                            nt = 2 * g + i
                            nc.tensor.matmul(
                                kp[0:osz, i * 512:(i + 1) * 512],
                                wv_[:, :, o0:o0 + osz],
                                cv[:, :, nt * 512:(nt + 1) * 512],
                                start=(part == 0), stop=(part == 1),
                                perf_mode=DR,
                            )
                    for i in range(2):
                        nt = 2 * g + i
                        tmp = tmp_pool.tile([128, 512], BF16, tag="tmp")
                        qb = q_sb[0:osz,
                                  mo * 1024 + blk * 128:mo * 1024 + (blk + 1) * 128]
                        qb = qb.unsqueeze(1).to_broadcast((osz, 4, 128))
                        nc.vector.tensor_mul(
                            tmp[0:osz, :].rearrange("c (d p) -> c d p", d=4),
                            kp[0:osz, i * 512:(i + 1) * 512].rearrange(
                                "c (d p) -> c d p", d=4),
                            qb,
                        )
                        tmp_tiles[(nt, mo)] = tmp

            def selgrp(g):
                for i in range(2):
                    nt = 2 * g + i
                    sim = sp_pool.tile([8, 512], F32, tag="sp")
                    for mo, (o0, osz) in enumerate(CHUNKS):
                        nc.tensor.matmul(
                            sim[0:8, :],
                            sel_sb[0:osz, mo * 8:mo * 8 + 8],
                            tmp_tiles.pop((nt, mo))[0:osz, :],
                            start=(mo == 0), stop=(mo == 2),
                        )
                    nc.scalar.activation(s8[0:8, nt * 512:(nt + 1) * 512],
                                         sim[0:8, :],
                                         mybir.ActivationFunctionType.Exp)

            kgrp(0)
            kgrp(1)
            selgrp(0)
            kgrp(2)
            selgrp(1)
            kgrp(3)
            selgrp(2)
            selgrp(3)

            # broadcast attn rows 8 -> 320 via DRAM bounce.  Issued from
            # the sync ring: the compute-gated store/bcast only delays the
            # next block's small fp8 loads (which have a full step of
            # slack); gpsimd must stay DMA-free (folds + SWDGE racing),
            # and on scalar these would block the vpd drains.
            sc = s8d_t.ap()[blk % 2]
            nc.sync.dma_start(sc, s8[0:8, :])
            ebc_sb = [ebc_pool.tile([128, DP], BF16, tag="ebc",
                                    name=f"ebc{mo}")
                      for mo in range(3)]
            for mo, (o0, osz) in enumerate(CHUNKS):
                nv = 64 if mo == 2 else VSZ[mo]
                for (r0, h0, nh, reps) in _bcast_runs(o0, nv):
                    src = sc[h0:h0 + nh, :].unsqueeze(1).to_broadcast(
                        (nh, reps, DP))
                    nc.sync.dma_start(ebc_sb[mo][r0:r0 + nh * reps, :], src)
            nc.sync.dma_start(ebc_sb[2][64:72, :], sc)  # raw attn for den
            return ctx_sb, ebc_sb

        def phase_b1(blk, ctx_sb, ebc_sb):
            """V projection (bf16, 2-nt PSUM groups), ACT drain, v*attn
            (DVE 2x), d-fold: step1 DVE, steps 2-5 GPSIMD."""
            mv_sb = [mv_pool.tile([128, DP], BF16, tag="mv", name=f"mv{mo}")
                     for mo in range(3)]
            for g in range(4):
                for mo, (o0, osz) in enumerate(CHUNKS):
                    vsz = VSZ[mo]
                    vp = pp_pool.tile([128, 1024], F32, tag="pp")
                    for kc, (c0, csz) in enumerate(CHUNKS):
                        ksz = csz + 1 if kc == 2 else csz
                        for i in range(2):
                            nt = 2 * g + i
                            nc.tensor.matmul(
                                vp[0:vsz, i * 512:(i + 1) * 512],
                                wv_sb[0:ksz, kc * 328 + o0:kc * 328 + o0 + vsz],
                                ctx_sb[kc][0:ksz, nt * 512:(nt + 1) * 512],
                                start=(kc == 0), stop=(kc == 2),
                            )
                    for i in range(2):
                        nt = 2 * g + i
                        vpd = vpd_pool.tile([128, 512], BF16, tag="vpd")
                        nc.scalar.activation(vpd[0:vsz, :],
                                             vp[0:vsz, i * 512:(i + 1) * 512],
                                             mybir.ActivationFunctionType.Copy)
                        nc.vector.tensor_mul(
                            mv_sb[mo][0:vsz, nt * 512:(nt + 1) * 512],
                            vpd[0:vsz, :],
                            ebc_sb[mo][0:vsz, nt * 512:(nt + 1) * 512],
                        )

            # d-reduce: fold 32 -> 1 over the outer free dim (d-major, so
            # every operand keeps a stride-1 innermost pix dim)
            ov_sb = ov_pool.tile([128, 384], F32, tag="ov")
            for mo in range(3):
                vsz = VSZ[mo]
                fs = fs_pool.tile([128, 3840], BF16, tag="fs")
                src = mv_sb[mo][0:vsz, :].rearrange("c (d p) -> c d p", d=D)
                o1 = fs[0:vsz, 0:2048].rearrange("c (d p) -> c d p", d=16)
                nc.vector.tensor_add(o1, src[:, 0:16, :], src[:, 16:32, :])
                cur = o1
                w = 16
                for off in (2048, 3072, 3584):
                    w //= 2
                    dst = fs[0:vsz, off:off + w * 128].rearrange(
                        "c (d p) -> c d p", d=w)
                    nc.gpsimd.tensor_add(dst, cur[:, 0:w, :], cur[:, w:2 * w, :])
                    cur = dst
                nc.gpsimd.tensor_add(
                    ov_sb[0:vsz, mo * 128:(mo + 1) * 128].rearrange(
                        "c (d p) -> c d p", d=1),
                    cur[:, 0:1, :], cur[:, 1:2, :])
            return ov_sb

        def phase_b2(blk, ov_sb):
            """Reciprocal, normalize, output projection, DMA out."""
            b = blk // (PIX_B // P)
            p0 = (blk % (PIX_B // P)) * P
            hr = p0 // W
            nh = P // W

            r8_sb = sm_pool.tile([128, P], F32R, tag="r8")
            with nc.allow_low_precision(reason="f32r reciprocal feeding matmul"):
                nc.vector.reciprocal(r8_sb[64:72, :], ov_sb[64:72, 2 * P:3 * P])
            att_sb = sm_pool.tile([128, 384], BF16, tag="att")
            r_ps = mp_pool.tile([128, 512], F32, tag="mp")
            for mo, (o0, osz) in enumerate(CHUNKS):
                nc.tensor.matmul(
                    r_ps[0:osz, mo * P:mo * P + P],
                    rsel_sb[64:72, mo * 128:mo * 128 + osz],
                    r8_sb[64:72, :],
                )
            # one normalize multiply over all three chunks; rows past each
            # chunk's VSZ are junk x junk and never read by the y-projection
            nc.vector.tensor_mul(
                att_sb[0:128, 0:384],
                ov_sb[0:128, 0:384],
                r_ps[0:128, 0:384],
            )

            y_ps = pp_pool.tile([128, 1024], F32, tag="pp")
            for mo, (o0, osz) in enumerate(CHUNKS):
                for kc, (c0, csz) in enumerate(CHUNKS):
                    nc.tensor.matmul(
                        y_ps[0:osz, mo * P:mo * P + P],
                        wo_sb[0:csz, kc * 320 + o0:kc * 320 + o0 + osz],
                        att_sb[0:csz, kc * P:kc * P + P],
                        start=(kc == 0), stop=(kc == 2),
                    )
            y_sb = y_pool.tile([128, 384], F32, tag="y")
            for mo, (o0, osz) in enumerate(CHUNKS):
                nc.scalar.add(
                    y_sb[0:osz, mo * P:mo * P + P],
                    y_ps[0:osz, mo * P:mo * P + P],
                    bout_sb[0:osz, mo:mo + 1],
                )
            for mo, (o0, osz) in enumerate(CHUNKS):
                dst = out_ap[b, o0:o0 + osz, hr:hr + nh, :].rearrange(
                    "c h w -> c (h w)")
                nc.scalar.dma_start(dst, y_sb[0:osz, mo * P:mo * P + P])

        # software pipeline, depth 2: A(s) | B2(s-1) | B1(s).  B1(s) runs
        # in the SAME step as A(s): the v-projection's ~19us of PE work
        # does not depend on the attn broadcast, so it covers the ~9us
        # bounce+bcast latency while keeping the PE dense (HAM warm) and
        # ctx prefetched exactly one block ahead.  B2 sits between so its
        # short DVE ops (recip/norm) queue ahead of B1's mul/fold chain.
        st_b = {}
        for s in range(NBLK + 1):
            if s < NBLK:
                ctx_sb, ebc_sb = phase_a(s)
            if s >= 1:
                phase_b2(s - 1, st_b.pop(s - 1))
            if s < NBLK:
                st_b[s] = phase_b1(s, ctx_sb, ebc_sb)

    nc.compile()
    return nc


_CACHED = {}


def _get_nc():
    if "nc" not in _CACHED:
        _CACHED["nc"] = build_nc()
    return _CACHED["nc"]


def make_core_inputs(x, context, wq, wk, wv, wout, bout):
    """Full inputs -> list of 8 per-core input dicts (host prep: shard,
    block, d-major layout, ones-channel, bf16 + fp8 DR panels)."""
    sel, rsel = make_constants()
    wk8m, wk8t = pack_wk_fp8(wk)
    consts = {
        "wq_p": pack_weight_T(np.asarray(wq, np.float32) * (SCALE / WK_SC)),
        "wk8m_p": wk8m, "wk8t_p": wk8t,
        "wv_p": pack_weight_T(np.asarray(wv, np.float32), ones_cols=True),
        "wo_p": pack_weight_T(np.asarray(wout, np.float32)),
        "sel_p": sel, "rsel_p": rsel,
    }
    bout_p = np.zeros((128, 3), np.float32)
    for mo, (o0, osz) in enumerate(CHUNKS):
        bout_p[0:osz, mo] = np.asarray(bout, np.float32)[o0:o0 + osz]
    consts["bout_p"] = bout_p
    x = np.asarray(x, np.float32)
    context = np.asarray(context, np.float32)
    nbh = PIX_B // P  # 4
    in_maps = []
    for cid in range(NCORES):
        h0 = cid * HLOC
        cs = context[:, :, :, h0:h0 + HLOC, :]  # [B, C, D, HLOC, W]
        # d-major: free = (d, pix), pix innermost
        cs = cs.reshape(B, CIN, D, nbh, P).transpose(0, 3, 1, 2, 4)
        cs = np.ascontiguousarray(cs.reshape(NBLK, CIN, DP), np.float32)
        panel = np.ones((NBLK, CIN + 1, DP), NPBF)
        panel[:, 0:CIN, :] = cs.astype(NPBF)
        # fp8 DoubleRow planar-pair panels: main c = j*128+p, tail c = 256+j*32+p
        c8 = to_fp8(cs)
        c8m = np.empty((NBLK, 128, 2 * DP), NPF8)
        c8m[:, :, 0:DP] = c8[:, 0:128, :]
        c8m[:, :, DP:2 * DP] = c8[:, 128:256, :]
        c8t = np.empty((NBLK, 32, 2 * DP), NPF8)
        c8t[:, :, 0:DP] = c8[:, 256:288, :]
        c8t[:, :, DP:2 * DP] = c8[:, 288:320, :]
        # x: [CIN, (blk, pix)]
        xs = x[:, :, h0:h0 + HLOC, :].reshape(B, CIN, nbh, P).transpose(1, 0, 2, 3)
        xs = np.ascontiguousarray(xs.reshape(CIN, NBLK * P), dtype=NPBF)
        m = dict(consts)
        m["ctx"] = panel
        m["c8m"] = c8m
        m["c8t"] = c8t
        m["x"] = xs
        in_maps.append(m)
    return in_maps


def kernel(x, context, wq, wk, wv, wout, bout):
    from concourse.bass_utils import run_bass_kernel_spmd

    nc = _get_nc()
    in_maps = make_core_inputs(x, context, wq, wk, wv, wout, bout)
    res = run_bass_kernel_spmd(nc, in_maps, list(range(NCORES)))
    shards = [res.results[c]["out"] for c in range(NCORES)]
    return np.concatenate(shards, axis=2).astype(np.float32)


if __name__ == "__main__":
    nc = build_nc()
    print("build + compile OK")
